# revision 11
# baseline (speedup 1.0000x reference)
"""Trainium2 Bass kernel for a 2-layer LSTM text classifier.

Model: embedding lookup -> 2-layer BasicLSTM (H=100, T=25) -> dense(128)
-> dense(2). Batch 512 is data-parallel across 8 NeuronCores (64
rows/core); parameters are replicated. The embedding gather happens
host-side so only the used rows ship, pre-transposed to feature-major
layout. fc1 and fc2 are both linear, so they are folded host-side into a
single [100, 2] head matmul.

Device kernel design notes:
- Feature-major layout everywhere: [hidden=100 partitions, batch=64
  free]; the recurrence never needs a transpose.
- All four gates go through a single tanh activation per cell:
  sigmoid(x) = (1 + tanh(x/2))/2, with the 1/2 prescale folded into
  the i/f/o weight columns host-side. Keeping the cell state doubled
  (C = 2c) makes the cell update exactly 4 scalar_tensor_tensor ops +
  2 activations.
- Input DMA: everything ships in ONE packed dram tensor; three
  partition-third DMAs (sync/scalar/gpsimd queues, in parallel) carry
  the xt head (t<6) plus all weights, three more carry the xt tail.
  Separate SBUF tiles keep the tail DMAs off the t<6 critical path.
- Biases: z1 bias rides in w1x's row 100 against xt's ones row; z2 bias
  enters via a rank-4 selector matmul; head bias via a ones-vector
  matmul.
"""

import functools
import os
import sys

import numpy as np

for _p in ("/opt/trn_rl_repo", "/root/.axon_site/_ro/trn_rl_repo"):
    if os.path.isdir(_p) and _p not in sys.path:
        sys.path.insert(0, _p)
        break

import ml_dtypes

from concourse import bass, bass2jax, mybir
from concourse.bass_utils import run_bass_kernel_spmd
from concourse.tile import TileContext

# --- BIR sync-wait rebalancer -------------------------------------------
# The walrus build in this image enforces ONE sync-wait command per ISA
# instruction struct, but Tile's semaphore assignment happily emits 2-4
# waits on matmuls/DVE ops at psum-recycle points. Rewrite the BIR before
# walrus: park one matmul wait on the adjacent Ldweights (same in-order
# queue, executes strictly before the matmul) and split any remaining
# excess onto pure-wait EventSemaphore carriers inserted directly before
# the offending instruction on its own queue. Semantics are unchanged --
# every wait still completes before the instruction it guarded.

_WAIT_PASSTHROUGH = {"EventSemaphore", "UnconditionalBranch", "Call",
                     "RegisterMove", "ISA"}


def _rebalance_bir_waits(bir_bytes):
    import orjson
    bir = orjson.loads(bir_bytes)
    n = 0
    for fn in bir["functions"]:
        for blk in fn["blocks"]:
            out = []
            prev = None
            for inst in blk["instructions"]:
                op = inst.get("opcode")
                si = inst.get("sync_info") or {}
                waits = si.get("on_wait") or []
                if op not in _WAIT_PASSTHROUGH and len(waits) > 1:
                    if (op == "Matmult" and prev is not None
                            and prev.get("opcode") == "Ldweights"
                            and not (prev.get("sync_info") or {}).get("on_wait")):
                        tsi = prev.setdefault("sync_info", {})
                        tsi.setdefault("on_wait", []).append(waits.pop(0))
                    while len(waits) > 1:
                        n += 1
                        out.append({
                            "debug": inst.get("debug", 0),
                            "engine": inst["engine"],
                            "ins": [], "outs": [],
                            "name": f"antwait_{n}",
                            "opcode": "EventSemaphore",
                            "sync_info": {"on_update": [],
                                          "on_wait": [waits.pop(0)]},
                        })
                    si["on_wait"] = waits
                out.append(inst)
                prev = inst
            blk["instructions"] = out
    return orjson.dumps(bir)


_orig_compile_bir_kernel = bass2jax.compile_bir_kernel


def _compile_bir_kernel_rebalanced(bir_json, tmpdir, neff_name="file.neff"):
    return _orig_compile_bir_kernel(_rebalance_bir_waits(bir_json), tmpdir,
                                    neff_name=neff_name)


if bass2jax.compile_bir_kernel is not _compile_bir_kernel_rebalanced:
    bass2jax.compile_bir_kernel = _compile_bir_kernel_rebalanced

H = 100          # hidden size
T = 25           # sequence length
B = 512          # total batch
N_CORES = 8
BC = B // N_CORES  # 64 per-core batch
FC = 128         # fc1 width (folded away on device)
NCLS = 2         # logits
FORGET_BIAS = 1.0

T_HEAD = 6       # timesteps shipped in DMA phase 1
XH = T_HEAD * BC          # 384 cols of xt head
XT_ALL = T * BC           # 1600
XTAIL = XT_ALL - XH       # 1216
WCOLS = 2436              # packed weight columns
# pk layout: [0:XH] xt head | [XH:XH+WCOLS] weights | [XH+WCOLS:] xt tail
PK1 = XH + WCOLS          # phase-1 cols
PKCOLS = PK1 + XTAIL

# weight sub-ranges within the weight block (offset by XH in pk)
W1X0, W1H0, W2X0, W2H0 = 0, 512, 1024, 1536
B2T0, B2SEL0, WHD0, BHD0 = 2048, 2176, 2432, 2434

BF16 = ml_dtypes.bfloat16
_DT = mybir.dt
TANH = mybir.ActivationFunctionType.Tanh
ADD = mybir.AluOpType.add
MULT = mybir.AluOpType.mult

# gate slot order in PSUM: [i, f, o, j]; source block order in the
# TF BasicLSTMCell kernel is [i, j, f, o]
SLOT_SRC_BLOCK = (0, 2, 3, 1)
SLOT_PRESCALE = (0.5, 0.5, 0.5, 1.0)  # tanh(x/2) trick for i/f/o, plain tanh for j


def _build_nc():
    nc = bass.Bass()
    pk_d = nc.dram_tensor("pk", [H + 1, PKCOLS], _DT.bfloat16, kind="ExternalInput")
    out_d = nc.dram_tensor("out", [NCLS, BC], _DT.float32, kind="ExternalOutput")

    with TileContext(nc) as tc:
        with tc.tile_pool(name="const", bufs=1) as cpool, \
             tc.tile_pool(name="work", bufs=26) as wpool, \
             tc.tile_pool(name="psum", bufs=3, space="PSUM") as ppool, \
             tc.tile_pool(name="psfc", bufs=1, space="PSUM") as fpool:

            # warm the tanh table on ACT while DMAs run
            scratch = cpool.tile([1, 1], _DT.float32, tag="scratch")
            nc.vector.memset(scratch[:, :], 0.0)
            nc.scalar.activation(scratch[:, :], scratch[:, :], TANH)

            pk1 = cpool.tile([H + 1, PK1], _DT.bfloat16, tag="pk1")
            pk2 = cpool.tile([H + 1, XTAIL], _DT.bfloat16, tag="pk2")
            ones = cpool.tile([1, BC], _DT.bfloat16, tag="ones")

            # SWDGE (gpsimd) DMAs of ~34 rows spread their descriptors
            # across many DMA engines (measured ~7us for 531KB); one big
            # 101-row DMA pins to 1-2 engines (~24us). Phase 1: xt head +
            # all weights as three gpsimd thirds; phase 2: xt tail (only
            # gates t >= T_HEAD) on the sync/scalar HWDGE queues.
            nc.gpsimd.dma_start(out=pk1[0:34, :], in_=pk_d[0:34, 0:PK1])
            nc.gpsimd.dma_start(out=pk1[34:68, :], in_=pk_d[34:68, 0:PK1])
            nc.gpsimd.dma_start(out=pk1[68:101, :], in_=pk_d[68:101, 0:PK1])

            nc.vector.memset(ones[:, :], 1.0)

            w = pk1[:, XH:XH + WCOLS]
            w1x = w[:, W1X0:W1X0 + 512]            # [101, 512] incl bias row
            w1h = w[0:H, W1H0:W1H0 + 512]
            w2x = w[0:H, W2X0:W2X0 + 512]
            w2h = w[0:H, W2H0:W2H0 + 512]
            b2t = w[0:4, B2T0:B2T0 + 128]
            b2sel = w[0:4, B2SEL0:B2SEL0 + 256]
            whd = w[0:H, WHD0:WHD0 + 2]
            bhd = w[0:1, BHD0:BHD0 + 2]

            def xslice(t):
                if t < T_HEAD:
                    return pk1[0:H + 1, t * BC:(t + 1) * BC]
                return pk2[0:H + 1, (t - T_HEAD) * BC:(t - T_HEAD + 1) * BC]

            # recurrent state; [H, BC], ping-pong; slot 1 is the t=0 input
            h1 = [cpool.tile([H, BC], _DT.bfloat16, tag=f"h1_{i}", name=f"h1_{i}") for i in range(2)]
            h2 = [cpool.tile([H, BC], _DT.bfloat16, tag=f"h2_{i}", name=f"h2_{i}") for i in range(2)]
            c1 = [cpool.tile([H, BC], _DT.float32, tag=f"c1_{i}", name=f"c1_{i}") for i in range(2)]
            c2 = [cpool.tile([H, BC], _DT.float32, tag=f"c2_{i}", name=f"c2_{i}") for i in range(2)]
            nc.vector.memset(h1[1][:, :], 0.0)
            nc.vector.memset(h2[1][:, :], 0.0)
            nc.vector.memset(c1[1][:, :], 0.0)
            nc.vector.memset(c2[1][:, :], 0.0)

            def x_part(z1, t):
                # layer-1 input contribution + bias; first write of the bank
                for g in range(4):
                    nc.tensor.matmul(
                        z1[0:128, g * 64:(g + 1) * 64],
                        lhsT=w1x[0:H + 1, g * 128:(g + 1) * 128],
                        rhs=xslice(t),
                        start=(g == 0), stop=False)

            def z2_open(z2, h2_rd):
                # bias broadcast via one-hot selector, then the (slack-rich)
                # h2 recurrent contribution
                nc.tensor.matmul(z2[0:128, 0:256], lhsT=b2t[0:4, 0:128],
                                 rhs=b2sel[0:4, 0:256], start=True, stop=False)
                for g in range(4):
                    nc.tensor.matmul(
                        z2[0:128, g * 64:(g + 1) * 64],
                        lhsT=w2h[0:H, g * 128:(g + 1) * 128],
                        rhs=h2_rd[0:H, :],
                        start=False, stop=False)

            def cell(z, c_rd, c_wr, h_wr, tag):
                # gates -> new cell state / hidden, all in [H, BC] layout
                tg = wpool.tile([128, 256], _DT.bfloat16, tag=f"t{tag}", name=f"t{tag}")
                nc.scalar.activation(tg[0:H, :], z[0:H, 0:256], TANH)
                ti, tf = tg[0:H, 0:64], tg[0:H, 64:128]
                to, tj = tg[0:H, 128:192], tg[0:H, 192:256]
                qh = wpool.tile([H, BC], _DT.float32, tag=f"q{tag}", name=f"q{tag}")
                nc.vector.scalar_tensor_tensor(qh[:, :], tf, 1.0, c_rd[:, :], op0=ADD, op1=MULT)
                ph = wpool.tile([H, BC], _DT.bfloat16, tag=f"p{tag}", name=f"p{tag}")
                nc.vector.scalar_tensor_tensor(ph[:, :], ti, 1.0, tj, op0=ADD, op1=MULT)
                nc.vector.scalar_tensor_tensor(c_wr[:, :], qh[:, :], 0.5, ph[:, :], op0=MULT, op1=ADD)
                tcg = wpool.tile([H, BC], _DT.bfloat16, tag=f"tc{tag}", name=f"tc{tag}")
                nc.scalar.activation(tcg[:, :], c_wr[:, :], TANH, scale=0.5)
                nc.vector.scalar_tensor_tensor(h_wr[0:H, :], to, 1.0, tcg[:, :], op0=ADD, op1=MULT)

            def w1h_close(z1, h1_rd):
                for g in range(4):
                    nc.tensor.matmul(
                        z1[0:128, g * 64:(g + 1) * 64],
                        lhsT=w1h[0:H, g * 128:(g + 1) * 128],
                        rhs=h1_rd[0:H, :],
                        start=False, stop=(g == 3))

            # software-pipelined prologue: z1(0) fully formed, z2(0) opened
            z1 = ppool.tile([128, 256], _DT.float32, tag="z1")
            x_part(z1, 0)
            w1h_close(z1, h1[1])
            z2 = ppool.tile([128, 256], _DT.float32, tag="z2")
            z2_open(z2, h2[1])

            # delayed copy of h1 for the W2x matmuls: it becomes ready one
            # DVE op after h1 itself, so the scheduler always runs the
            # critical W1h(t+1) matmuls first on the in-order PE queue
            h1c = [cpool.tile([H, BC], _DT.bfloat16, tag=f"h1c_{i}", name=f"h1c_{i}") for i in range(2)]
            nc.vector.memset(h1c[1][:, :], 0.0)

            for t in range(T):
                rd, wr = (t + 1) % 2, t % 2
                if t == 1:
                    # issue the xt-tail DMAs only now so they don't contend
                    # with phase 1 for DMA engines
                    nc.sync.dma_start(out=pk2[0:50, :], in_=pk_d[0:50, PK1:PKCOLS])
                    nc.scalar.dma_start(out=pk2[50:101, :], in_=pk_d[50:101, PK1:PKCOLS])
                z1_cur = z1
                cell(z1_cur, c1[rd], c1[wr], h1[wr], "1")
                nc.vector.tensor_copy(h1c[wr][:, :], h1[wr][:, :])

                # IMMEDIATELY after cell1, form z1(t+1): its 4 W1h matmuls
                # are the only PE work between h1(t) and tanh(t+1), keeping
                # the layer-1 recurrence (the critical cycle) tight.
                if t + 1 < T:
                    z1 = ppool.tile([128, 256], _DT.float32, tag="z1")
                    x_part(z1, t + 1)
                    # highest scheduling priority: these 4 matmuls are the
                    # only PE work on the h1(t) -> tanh(t+1) critical cycle
                    with tc.high_priority():
                        w1h_close(z1, h1[wr])

                # layer 2: h1 contribution closes the z2 accumulation
                for g in range(4):
                    nc.tensor.matmul(
                        z2[0:128, g * 64:(g + 1) * 64],
                        lhsT=w2x[0:H, g * 128:(g + 1) * 128],
                        rhs=h1c[wr][0:H, :],
                        start=False, stop=(g == 3))
                cell(z2, c2[rd], c2[wr], h2[wr], "2")

                if t + 1 < T:
                    z2 = ppool.tile([128, 256], _DT.float32, tag="z2")
                    z2_open(z2, h2[wr])

            # classifier head (fc1@fc2 pre-folded) on the final h2
            last = (T - 1) % 2
            predp = fpool.tile([128, BC], _DT.float32, tag="pred", name="predp")
            nc.tensor.matmul(predp[0:NCLS, :], lhsT=whd[0:H, 0:NCLS],
                             rhs=h2[last][0:H, :], start=True, stop=False)
            nc.tensor.matmul(predp[0:NCLS, :], lhsT=bhd[0:1, 0:NCLS],
                             rhs=ones[0:1, :], start=False, stop=True)
            outs = wpool.tile([NCLS, BC], _DT.float32, tag="outs")
            nc.vector.tensor_copy(outs[:, :], predp[0:NCLS, :])
            nc.sync.dma_start(out=out_d[:, :], in_=outs[:, :])

    return nc


@functools.lru_cache(maxsize=1)
def _get_nc():
    return _build_nc()


def _scaled_gate_blocks(kmat, rows, extra_scale):
    """[rows x 512] tile: gate blocks reordered to [i,f,o,j], padded
    100->128 cols, prescaled for the tanh-only gate trick."""
    out = np.zeros((rows.stop - rows.start, 512), np.float32)
    for slot in range(4):
        b = SLOT_SRC_BLOCK[slot]
        out[:, slot * 128:slot * 128 + H] = (
            kmat[rows, b * H:(b + 1) * H] * (SLOT_PRESCALE[slot] * extra_scale))
    return out


def _prep_weights(k1, b1, k2, b2, w_fc1, b_fc1, w_fc2, b_fc2):
    wpack = np.zeros((H + 1, WCOLS), np.float32)
    wpack[0:H, W1X0:W1X0 + 512] = _scaled_gate_blocks(k1, slice(0, H), 1.0)
    b2tm = np.zeros((4, 128), np.float32)
    b2selm = np.zeros((4, 256), np.float32)
    for slot in range(4):
        b = SLOT_SRC_BLOCK[slot]
        fb = FORGET_BIAS if slot == 1 else 0.0
        wpack[H, W1X0 + slot * 128:W1X0 + slot * 128 + H] = (
            (b1[b * H:(b + 1) * H] + fb) * SLOT_PRESCALE[slot])
        b2tm[slot, 0:H] = (b2[b * H:(b + 1) * H] + fb) * SLOT_PRESCALE[slot]
        b2selm[slot, slot * 64:(slot + 1) * 64] = 1.0
    wpack[0:H, W1H0:W1H0 + 512] = _scaled_gate_blocks(k1, slice(H, 2 * H), 0.5)
    wpack[0:H, W2X0:W2X0 + 512] = _scaled_gate_blocks(k2, slice(0, H), 0.5)
    wpack[0:H, W2H0:W2H0 + 512] = _scaled_gate_blocks(k2, slice(H, 2 * H), 0.5)
    wpack[0:4, B2T0:B2T0 + 128] = b2tm
    wpack[0:4, B2SEL0:B2SEL0 + 256] = b2selm
    # fc1 and fc2 are both linear: fold into one [H, NCLS] head.
    # 0.5 compensates the doubled h (h~ = 2h).
    wpack[0:H, WHD0:WHD0 + 2] = 0.5 * (w_fc1 @ w_fc2)
    wpack[0:1, BHD0:BHD0 + 2] = (b_fc1 @ w_fc2 + b_fc2).reshape(1, NCLS)
    return wpack


def _run(inputs, trace=False):
    nc = _get_nc()
    feats = np.asarray(inputs["features"])
    x = np.asarray(inputs["embedding"])[feats]          # [B, T, H] host gather
    wpack = _prep_weights(
        np.asarray(inputs["k1"]), np.asarray(inputs["b1"]),
        np.asarray(inputs["k2"]), np.asarray(inputs["b2"]),
        np.asarray(inputs["w_fc1"]), np.asarray(inputs["b_fc1"]),
        np.asarray(inputs["w_fc2"]), np.asarray(inputs["b_fc2"]))
    in_maps = []
    for c in range(N_CORES):
        xt = np.ones((H + 1, XT_ALL), np.float32)
        # [BC, T, H] -> [H, T, BC] feature-major with a trailing ones row
        xt[0:H] = x[c * BC:(c + 1) * BC].transpose(2, 1, 0).reshape(H, XT_ALL)
        pk = np.empty((H + 1, PKCOLS), np.float32)
        pk[:, 0:XH] = xt[:, 0:XH]
        pk[:, XH:XH + WCOLS] = wpack
        pk[:, PK1:PKCOLS] = xt[:, XH:XT_ALL]
        in_maps.append({"pk": pk.astype(BF16)})
    res = run_bass_kernel_spmd(nc, in_maps, core_ids=list(range(N_CORES)),
                               trace=trace)
    out = np.empty((B, NCLS), np.float32)
    for c in range(N_CORES):
        out[c * BC:(c + 1) * BC] = res.results[c]["out"].T
    return out, res


def kernel(**inputs):
    out, _ = _run(inputs, trace=False)
    return out
